# revision 55
# baseline (speedup 1.0000x reference)
"""Trainium2 kernel for nn_ContConv1dDense (banded continuous conv with
kernel-MLP), data-parallel over (batch, sequence-half) on 8 NeuronCores.

Math: the reference computes, per (b, i, k in 1..8):
    dt      = (times[b,i] - times[b,i-k]) masked to the band & valid length
    hidden  = relu(dt * W1 + b1)                       # (128,)
    kv      = (hidden @ W2 + b2).reshape(32, 32)       # masked
    out[b,i,:] += features[b,i-k,:] @ kv

For this operator's input family, `times` is sorted (dt >= 0) and b1 == b2
== 0, so relu(dt*W1) == dt*max(W1,0) exactly and the kernel-MLP collapses
to a constant 32x32 matrix V = (max(W1,0) @ W2).reshape(32,32):

    out[i,:] = sum_k dt_m[i,k] * (features[i-k,:] @ V) = (A @ features @ V)[i,:]

where A is the 1024x1032 banded matrix of masked dt values.  Verified at
runtime by a guard; a numpy fallback handles general inputs.

Device program (core = 2*b + half, 1024 positions each):
  Stride-120 tiling: tile t covers h-positions Hs_t = start-8+120t ..
  +127; output rows 8..127 of each tile are valid (the 8-wide band then
  never crosses a tile boundary -- no halo matmuls).
  Stage 1 (PE): h_t = fT_t^T @ V            (9 matmuls, fT host-transposed)
  DVE: one strided cast drains h (3 PSUM banks, f32) -> hS fp16
  Stage 2 (PE): out_t = A_t @ h_t as two halves per tile from the
  host-built banded-dt stationaries ATa [64x64] / ATb [128x64]
  DVE: 2 copies out-PSUM -> osb fp16; one fp16 output DMA (host upcasts).
  Everything is host-packed into SBUF-image layouts (contiguous
  per-partition descriptors) and spread over 3 DMA queues (Sync, Scalar,
  GpSimd) because a single queue sustains only ~60-120 B/ns; the
  h-gating DMA always issues first on the shared HWDGE.
"""

import numpy as np

KS = 8          # band width (kernel size)
B = 4
L = 2048
C = 32          # in channels
OUT = 32        # out channels
HALF = 1024     # positions per core
NT = 9          # stride-120 tiles per core
STRIDE = 120
N_CORES = 8

# imgA fp16 column layout: [fTs 384 | V 32] = 416
FT_W = 384
V_OFF = 384
IMGA_W = 416
# banded-split AT: ATa [64q x 64r] per tile (out rows 0..63; band keeps
# q<=62 there), ATb [128q x 64r] per tile (out rows 64..127)
ATA_W = NT * 64   # 576 cols, 64 partitions
ATB_W = NT * 64   # 576 cols, 128 partitions
OSB_W = NT * OUT  # 288

_CACHE = {}


def _build_program_v3():
    from contextlib import ExitStack

    import concourse.bacc as bacc
    import concourse.bass as bass  # noqa: F401
    from concourse import mybir

    f32 = mybir.dt.float32
    f16 = mybir.dt.float16

    nc = bacc.Bacc(
        "TRN2", target_bir_lowering=False, debug=False, num_devices=N_CORES
    )

    # partial-partition DRAM images: fTs/V only use partitions 0-95, and
    # ATb rows q<56 are structurally zero (band: out rows 64-127 reach back
    # to q=56 at most) -- don't ship dead partitions
    inA = nc.dram_tensor("inA", [96, IMGA_W], f16, kind="ExternalInput").ap()
    inBa = nc.dram_tensor("inBa", [64, ATA_W], f16, kind="ExternalInput").ap()
    inBb = nc.dram_tensor("inBb", [72, ATB_W], f16, kind="ExternalInput").ap()
    # fp16 output (host upcasts): halves the final DMA the teardown drains
    out = nc.dram_tensor("out", [128, OSB_W], f16, kind="ExternalOutput").ap()

    imgA = nc.alloc_sbuf_tensor("imgA", [128, IMGA_W], f16).ap()
    ATa = nc.alloc_sbuf_tensor("ATa", [64, ATA_W], f16).ap()
    ATb = nc.alloc_sbuf_tensor("ATb", [128, ATB_W], f16).ap()
    O_SPLIT = 4  # o-tiles 0..3 -> PSUM bank 0 / copy chunk 1; 4..8 -> bank 1
    hS = nc.alloc_sbuf_tensor("hS", [128, OSB_W], f16).ap()
    osb = nc.alloc_sbuf_tensor("osb", [128, OSB_W], f16).ap()

    # psHall spans 3 PSUM banks (matmuls with different row-group
    # tile_positions must land in different banks -- HW constraint); all 9
    # o-tiles share one bank (same row group)
    psHall = nc.alloc_psum_tensor("psHall", [128, 1536], f32).ap()
    psOall = nc.alloc_psum_tensor("psOall", [128, 1024], f32).ap()

    def o_col(t):
        # o-tile PSUM column: chunk 1 in bank 0, chunk 2 in bank 1
        return OUT * t if t < O_SPLIT else 512 + OUT * (t - O_SPLIT)

    with ExitStack() as _sctx:
        block = _sctx.enter_context(nc.Block(no_gpsimd_drain=True))
        _names = ["sIN1", "sIN2a", "sIN2b", "sIN2c", "sGO", "sMZ", "sH",
                  "sHS", "sO", "sOS", "sOUT"]
        _sems = {n: _sctx.enter_context(nc.semaphore(n)) for n in _names}
        (sIN1, sIN2a, sIN2b, sIN2c, sGO, sMZ, sH, sHS, sO, sOS, sOUT) = (
            _sems[n] for n in _names
        )

        ATB_C = 64 * 5  # ATb tiles 0-4 ride Sync's queue; 5-8 via Scalar
        # (both pieces then complete ~simultaneously)

        @block.sync
        def _(sy):
            # HWDGE issue order: in1 (gates the h-chain), then Scalar's ATb
            # piece (its queue is empty, transfer starts immediately), then
            # Sync's ATb piece (its queue is busy with in1 anyway)
            sy.dma_start(imgA[0:96, :], inA[:]).then_inc(sIN1, 16)
            sy.sem_inc(sGO, 1)
            sy.wait_ge(sGO, 2)
            sy.dma_start(
                ATb[56:128, 0:ATB_C], inBb[:, 0:ATB_C]
            ).then_inc(sIN2c, 16)
            sy.wait_ge(sOS, 2)
            sy.dma_start(out[:], osb[:]).then_inc(sOUT, 16)
            # no completion wait: Sync's block-exit DRAIN covers the DMA

        @block.scalar
        def _(s):
            # own HW queue (a single queue sustains only ~60-120 B/ns, so
            # input DMAs must spread across queues); gated behind in1's issue
            s.wait_ge(sGO, 1)
            s.dma_start(
                ATb[56:128, ATB_C:ATB_W], inBb[:, ATB_C:ATB_W]
            ).then_inc(sIN2a, 16)
            s.sem_inc(sGO, 1)

        @block.gpsimd
        def _(g):
            g.dma_start(ATa[:], inBa[:]).then_inc(sIN2b, 16)

        @block.tensor
        def _(te):
            def h_mm(t):
                # tiles 3s+g: slot s = t//3 (row group 32s, PSUM bank s),
                # region g = t%3
                s, gg = t // 3, t % 3
                return nc.tensor.matmul(
                    psHall[:, 512 * s + OUT * gg : 512 * s + OUT * gg + OUT],
                    imgA[32 * s : 32 * s + 32, 128 * gg : 128 * gg + 128],
                    imgA[32 * s : 32 * s + 32, V_OFF : V_OFF + OUT],
                    start=True, stop=True,
                )

            def o_mm_a(t):
                # output rows 0..63 of tile t: band there has q <= 62
                return nc.tensor.matmul(
                    psOall[0:64, o_col(t) : o_col(t) + OUT],
                    ATa[:, 64 * t : 64 * t + 64],
                    hS[0:64, OUT * t : OUT * t + OUT],
                    start=True, stop=True,
                )

            def o_mm_b(t):
                # output rows 64..127 of tile t (out partition base 64)
                return nc.tensor.matmul(
                    psOall[64:128, o_col(t) : o_col(t) + OUT],
                    ATb[:, 64 * t : 64 * t + 64],
                    hS[:, OUT * t : OUT * t + OUT],
                    start=True, stop=True,
                )

            te.wait_ge(sIN1, 16)
            for t in range(9):
                ins = h_mm(t)
            ins.then_inc(sH, 1)
            te.wait_ge(sHS, 1)
            te.wait_ge(sIN2b, 16)
            for t in range(NT):
                o_mm_a(t)
            te.wait_ge(sMZ, 1)
            te.wait_ge(sIN2c, 16)
            for t in range(O_SPLIT):
                ins = o_mm_b(t)
            ins.then_inc(sO, 1)
            o_mm_b(O_SPLIT)
            te.wait_ge(sIN2a, 16)
            for t in range(O_SPLIT + 1, NT):
                ins = o_mm_b(t)
            ins.then_inc(sO, 1)

        @block.vector
        def _(v):
            # zero ATb's never-DMAed rows once (matmul reads all 128)
            nc.vector.memset(ATb[0:56, :], 0.0).then_inc(sMZ, 1)
            v.wait_ge(sH, 1)
            # single strided cast drains all three h banks at once
            nc.vector.tensor_copy(
                hS.rearrange("p (b x) -> p b x", b=3),
                psHall.rearrange("p (b x) -> p b x", b=3)[:, :, 0:96],
            ).then_inc(sHS, 1)
            v.wait_ge(sO, 1)
            nc.vector.tensor_copy(
                osb[:, 0 : OUT * O_SPLIT], psOall[:, 0 : OUT * O_SPLIT]
            ).then_inc(sOS, 1)
            v.wait_ge(sO, 2)
            nc.vector.tensor_copy(
                osb[:, OUT * O_SPLIT : OSB_W],
                psOall[:, 512 : 512 + OUT * (NT - O_SPLIT)],
            ).then_inc(sOS, 1)

    nc.compile()
    return nc


def _get_program():
    if "nc" not in _CACHE:
        _CACHE["nc"] = _build_program_v3()
    return _CACHE["nc"]


def _fast_path_ok(times, b1, b2):
    # The linearization relu(dt*W1 + b1) == dt*max(W1,0) is exact iff
    # b1 == 0 and dt >= 0 (times sorted); b2 == 0 removes the bias term.
    if np.any(b1 != 0.0) or np.any(b2 != 0.0):
        return False
    if np.any(np.diff(times, axis=1) < 0.0):
        return False
    return True


def _reference_fallback(times, features, lengths, W1, b1, W2, b2):
    # Straight numpy transcription of the reference (general inputs).
    Bn, Ln = times.shape
    offsets = np.arange(1, KS + 1)
    idx = np.arange(Ln)[:, None] - offsets[None, :]
    in_band = idx >= 0
    idx_c = np.clip(idx, 0, Ln - 1)
    t_j = times[:, idx_c]
    dt = times[:, :, None] - t_j
    pos_i = np.arange(Ln)[None, :, None]
    mask = (
        in_band[None]
        & (idx_c[None] < lengths[:, None, None])
        & (pos_i <= lengths[:, None, None] - 1)
    )
    dt = np.where(mask, dt, 0.0).astype(np.float32)
    hidden = np.maximum(dt[..., None] * W1[0] + b1, 0.0)
    kv = (hidden @ W2 + b2).reshape(Bn, Ln, KS, C, OUT)
    kv = np.where(mask[..., None, None], kv, 0.0)
    feat_g = features[:, idx_c]
    return np.einsum("blkc,blkco->blo", feat_g, kv).astype(np.float32)


def _build_in_maps(times, features, lengths, W1, W2):
    # Fold the (now linear) kernel-MLP into one 32x32 matrix.
    v16 = (np.maximum(W1[0], 0.0) @ W2).reshape(C, OUT).astype(np.float16)

    q = np.arange(128)
    r = np.arange(128)
    tt = np.arange(NT)

    in_maps = []
    for core in range(N_CORES):
        b, half = core // 2, core % 2
        start = half * HALF
        t_b = times[b]
        f_b = features[b]
        ln = int(lengths[b])

        Hs = start - 8 + STRIDE * tt                       # (NT,)
        jpos = Hs[:, None] + q[None, :]                    # (NT, 128) j per (t, q)
        ipos = Hs[:, None] + r[None, :]                    # (NT, 128) i per (t, r)

        # fTs[32*(t%4)+c, 128*(t//4)+q] = f_b[jpos[t,q], c] (0 out of range)
        jc = np.clip(jpos, 0, L - 1)
        fv = f_b[jc]                                       # (NT, 128, C)
        fv = np.where(((jpos >= 0) & (jpos < L))[..., None], fv, 0.0)
        imgA_h = np.zeros((96, IMGA_W), np.float16)
        for t in range(NT):
            s, g = t // 3, t % 3
            imgA_h[32 * s : 32 * s + 32, 128 * g : 128 * g + 128] = (
                fv[t].T.astype(np.float16)
            )
        for s in range(3):
            imgA_h[32 * s : 32 * s + 32, V_OFF : V_OFF + OUT] = v16

        # AT[q, t, r]: k = r - q in [1, 8]; value = t_b[i] - t_b[j], masked
        k = r[None, None, :] - q[:, None, None]            # (128, 1, 128)
        jq = jpos.T[:, :, None]                            # (128 q, NT, 1)
        ir = ipos[None, :, :]                              # (1, NT, 128)
        valid = (
            (k >= 1) & (k <= KS)
            & (jq >= 0) & (jq < ln)
            & (ir <= ln - 1) & (ir < L) & (ir >= 0)
        )                                                  # (128, NT, 128)
        ic = np.clip(ipos, 0, L - 1)                       # (NT, 128)
        dtv = t_b[ic][None, :, :] - t_b[jc].T[:, :, None]  # (128, NT, 128)
        at = np.where(valid, dtv, 0.0).astype(np.float16)  # (128 q, NT, 128 r)

        # banded split: out rows r<64 only need q<=62 (ATa, 64 partitions);
        # rows r>=64 need q in [56,126] (ATb, full partitions)
        inBa_h = np.ascontiguousarray(at[0:64, :, 0:64].reshape(64, ATA_W))
        inBb_h = np.ascontiguousarray(at[56:128, :, 64:128].reshape(72, ATB_W))

        in_maps.append({"inA": imgA_h, "inBa": inBa_h, "inBb": inBb_h})
    return in_maps


def kernel(times, features, lengths, W1, b1, W2, b2):
    times = np.asarray(times, dtype=np.float32)
    features = np.asarray(features, dtype=np.float32)
    lengths = np.asarray(lengths)
    W1 = np.asarray(W1, dtype=np.float32)
    b1 = np.asarray(b1, dtype=np.float32)
    W2 = np.asarray(W2, dtype=np.float32)
    b2 = np.asarray(b2, dtype=np.float32)

    if not _fast_path_ok(times, b1, b2):
        return _reference_fallback(times, features, lengths, W1, b1, W2, b2)

    from concourse.bass_utils import run_bass_kernel_spmd

    nc = _get_program()
    in_maps = _build_in_maps(times, features, lengths, W1, W2)
    res = run_bass_kernel_spmd(nc, in_maps, core_ids=list(range(N_CORES)))

    out = np.empty((B, L, OUT), np.float32)
    for core in range(N_CORES):
        b, half = core // 2, core % 2
        start = half * HALF
        r = res.results[core]["out"].astype(np.float32)   # (128, 288)
        for t in range(NT):
            n_t = min(STRIDE, HALF - STRIDE * t)
            if n_t <= 0:
                break
            out[b, start + STRIDE * t : start + STRIDE * t + n_t, :] = (
                r[8 : 8 + n_t, OUT * t : OUT * t + OUT]
            )
    return out


# revision 56
# speedup vs baseline: 1.0647x; 1.0647x over previous
"""Trainium2 kernel for nn_ContConv1dDense (banded continuous conv with
kernel-MLP), data-parallel over (batch, sequence-half) on 8 NeuronCores.

Math: the reference computes, per (b, i, k in 1..8):
    dt      = (times[b,i] - times[b,i-k]) masked to the band & valid length
    hidden  = relu(dt * W1 + b1)                       # (128,)
    kv      = (hidden @ W2 + b2).reshape(32, 32)       # masked
    out[b,i,:] += features[b,i-k,:] @ kv

For this operator's input family, `times` is sorted (dt >= 0) and b1 == b2
== 0, so relu(dt*W1) == dt*max(W1,0) exactly and the kernel-MLP collapses
to a constant 32x32 matrix V = (max(W1,0) @ W2).reshape(32,32):

    out[i,:] = sum_k dt_m[i,k] * (features[i-k,:] @ V) = (A @ features @ V)[i,:]

where A is the 1024x1032 banded matrix of masked dt values.  Verified at
runtime by a guard; a numpy fallback handles general inputs.

Device program (core = 2*b + half, 1024 positions each):
  Stride-120 tiling: tile t covers h-positions Hs_t = start-8+120t ..
  +127; output rows 8..127 of each tile are valid (the 8-wide band then
  never crosses a tile boundary -- no halo matmuls).
  Stage 1 (PE): h_t = fT_t^T @ V            (9 matmuls, fT host-transposed)
  DVE: one strided cast drains h (3 PSUM banks, f32) -> hS fp16
  Stage 2 (PE): out_t = A_t @ h_t as two halves per tile from the
  host-built banded-dt stationaries ATa [64x64] / ATb [128x64]
  DVE: 2 copies out-PSUM -> osb fp16; one fp16 output DMA (host upcasts).
  Everything is host-packed into SBUF-image layouts (contiguous
  per-partition descriptors) and spread over 3 DMA queues (Sync, Scalar,
  GpSimd) because a single queue sustains only ~60-120 B/ns; the
  h-gating DMA always issues first on the shared HWDGE.
"""

import numpy as np

KS = 8          # band width (kernel size)
B = 4
L = 2048
C = 32          # in channels
OUT = 32        # out channels
HALF = 1024     # positions per core
NT = 9          # stride-120 tiles per core
STRIDE = 120
N_CORES = 8

# imgA fp16 column layout: [fTs 384 | V 32] = 416
FT_W = 384
V_OFF = 384
IMGA_W = 416
# banded-split AT: ATa [64q x 64r] per tile (out rows 0..63; band keeps
# q<=62 there), ATb [128q x 64r] per tile (out rows 64..127)
ATA_W = NT * 64   # 576 cols, 64 partitions
ATB_W = NT * 64   # 576 cols, 128 partitions
OSB_W = NT * OUT  # 288

_CACHE = {}


def _build_program_v3():
    from contextlib import ExitStack

    import concourse.bacc as bacc
    import concourse.bass as bass  # noqa: F401
    from concourse import mybir

    f32 = mybir.dt.float32
    f16 = mybir.dt.float16

    nc = bacc.Bacc(
        "TRN2", target_bir_lowering=False, debug=False, num_devices=N_CORES
    )

    # partial-partition DRAM images: fTs/V only use partitions 0-95, and
    # ATb rows q<56 are structurally zero (band: out rows 64-127 reach back
    # to q=56 at most) -- don't ship dead partitions
    inA = nc.dram_tensor("inA", [96, IMGA_W], f16, kind="ExternalInput").ap()
    inBa = nc.dram_tensor("inBa", [64, ATA_W], f16, kind="ExternalInput").ap()
    inBb = nc.dram_tensor("inBb", [128, ATB_W], f16, kind="ExternalInput").ap()
    # fp16 output (host upcasts): halves the final DMA the teardown drains
    out = nc.dram_tensor("out", [128, OSB_W], f16, kind="ExternalOutput").ap()

    imgA = nc.alloc_sbuf_tensor("imgA", [128, IMGA_W], f16).ap()
    ATa = nc.alloc_sbuf_tensor("ATa", [64, ATA_W], f16).ap()
    ATb = nc.alloc_sbuf_tensor("ATb", [128, ATB_W], f16).ap()
    O_SPLIT = 4  # o-tiles 0..3 -> PSUM bank 0 / copy chunk 1; 4..8 -> bank 1
    hS = nc.alloc_sbuf_tensor("hS", [128, OSB_W], f16).ap()
    osb = nc.alloc_sbuf_tensor("osb", [128, OSB_W], f16).ap()

    # psHall spans 3 PSUM banks (matmuls with different row-group
    # tile_positions must land in different banks -- HW constraint); all 9
    # o-tiles share one bank (same row group)
    psHall = nc.alloc_psum_tensor("psHall", [128, 1536], f32).ap()
    psOall = nc.alloc_psum_tensor("psOall", [128, 1024], f32).ap()

    def o_col(t):
        # o-tile PSUM column: chunk 1 in bank 0, chunk 2 in bank 1
        return OUT * t if t < O_SPLIT else 512 + OUT * (t - O_SPLIT)

    with ExitStack() as _sctx:
        block = _sctx.enter_context(nc.Block(no_gpsimd_drain=True))
        _names = ["sIN1", "sIN2a", "sIN2b", "sIN2c", "sGO", "sH", "sHS",
                  "sO", "sOS", "sOUT"]
        _sems = {n: _sctx.enter_context(nc.semaphore(n)) for n in _names}
        (sIN1, sIN2a, sIN2b, sIN2c, sGO, sH, sHS, sO, sOS, sOUT) = (
            _sems[n] for n in _names
        )

        ATB_C = 64 * 5  # ATb tiles 0-4 ride Sync's queue; 5-8 via Scalar
        # (both pieces then complete ~simultaneously)

        @block.sync
        def _(sy):
            # HWDGE issue order: in1 (gates the h-chain), then Scalar's ATb
            # piece (its queue is empty, transfer starts immediately), then
            # Sync's ATb piece (its queue is busy with in1 anyway)
            sy.dma_start(imgA[0:96, :], inA[:]).then_inc(sIN1, 16)
            sy.sem_inc(sGO, 1)
            sy.wait_ge(sGO, 2)
            sy.dma_start(ATb[:, 0:ATB_C], inBb[:, 0:ATB_C]).then_inc(
                sIN2c, 16
            )
            sy.wait_ge(sOS, 2)
            sy.dma_start(out[:], osb[:]).then_inc(sOUT, 16)
            # no completion wait: Sync's block-exit DRAIN covers the DMA

        @block.scalar
        def _(s):
            # own HW queue (a single queue sustains only ~60-120 B/ns, so
            # input DMAs must spread across queues); gated behind in1's issue
            s.wait_ge(sGO, 1)
            s.dma_start(ATb[:, ATB_C:ATB_W], inBb[:, ATB_C:ATB_W]).then_inc(
                sIN2a, 16
            )
            s.sem_inc(sGO, 1)

        @block.gpsimd
        def _(g):
            g.dma_start(ATa[:], inBa[:]).then_inc(sIN2b, 16)

        @block.tensor
        def _(te):
            def h_mm(t):
                # tiles 3s+g: slot s = t//3 (row group 32s, PSUM bank s),
                # region g = t%3
                s, gg = t // 3, t % 3
                return nc.tensor.matmul(
                    psHall[:, 512 * s + OUT * gg : 512 * s + OUT * gg + OUT],
                    imgA[32 * s : 32 * s + 32, 128 * gg : 128 * gg + 128],
                    imgA[32 * s : 32 * s + 32, V_OFF : V_OFF + OUT],
                    start=True, stop=True,
                )

            def o_mm_a(t):
                # output rows 0..63 of tile t: band there has q <= 62
                return nc.tensor.matmul(
                    psOall[0:64, o_col(t) : o_col(t) + OUT],
                    ATa[:, 64 * t : 64 * t + 64],
                    hS[0:64, OUT * t : OUT * t + OUT],
                    start=True, stop=True,
                )

            def o_mm_b(t):
                # output rows 64..127 of tile t (out partition base 64)
                return nc.tensor.matmul(
                    psOall[64:128, o_col(t) : o_col(t) + OUT],
                    ATb[:, 64 * t : 64 * t + 64],
                    hS[:, OUT * t : OUT * t + OUT],
                    start=True, stop=True,
                )

            te.wait_ge(sIN1, 16)
            for t in range(9):
                ins = h_mm(t)
            ins.then_inc(sH, 1)
            te.wait_ge(sHS, 1)
            te.wait_ge(sIN2b, 16)
            for t in range(NT):
                o_mm_a(t)
            te.wait_ge(sIN2c, 16)
            for t in range(O_SPLIT):
                ins = o_mm_b(t)
            ins.then_inc(sO, 1)
            o_mm_b(O_SPLIT)
            te.wait_ge(sIN2a, 16)
            for t in range(O_SPLIT + 1, NT):
                ins = o_mm_b(t)
            ins.then_inc(sO, 1)

        @block.vector
        def _(v):
            v.wait_ge(sH, 1)
            # single strided cast drains all three h banks at once
            nc.vector.tensor_copy(
                hS.rearrange("p (b x) -> p b x", b=3),
                psHall.rearrange("p (b x) -> p b x", b=3)[:, :, 0:96],
            ).then_inc(sHS, 1)
            v.wait_ge(sO, 1)
            nc.vector.tensor_copy(
                osb[:, 0 : OUT * O_SPLIT], psOall[:, 0 : OUT * O_SPLIT]
            ).then_inc(sOS, 1)
            v.wait_ge(sO, 2)
            nc.vector.tensor_copy(
                osb[:, OUT * O_SPLIT : OSB_W],
                psOall[:, 512 : 512 + OUT * (NT - O_SPLIT)],
            ).then_inc(sOS, 1)

    nc.compile()
    return nc


def _get_program():
    if "nc" not in _CACHE:
        _CACHE["nc"] = _build_program_v3()
    return _CACHE["nc"]


def _fast_path_ok(times, b1, b2):
    # The linearization relu(dt*W1 + b1) == dt*max(W1,0) is exact iff
    # b1 == 0 and dt >= 0 (times sorted); b2 == 0 removes the bias term.
    if np.any(b1 != 0.0) or np.any(b2 != 0.0):
        return False
    if np.any(np.diff(times, axis=1) < 0.0):
        return False
    return True


def _reference_fallback(times, features, lengths, W1, b1, W2, b2):
    # Straight numpy transcription of the reference (general inputs).
    Bn, Ln = times.shape
    offsets = np.arange(1, KS + 1)
    idx = np.arange(Ln)[:, None] - offsets[None, :]
    in_band = idx >= 0
    idx_c = np.clip(idx, 0, Ln - 1)
    t_j = times[:, idx_c]
    dt = times[:, :, None] - t_j
    pos_i = np.arange(Ln)[None, :, None]
    mask = (
        in_band[None]
        & (idx_c[None] < lengths[:, None, None])
        & (pos_i <= lengths[:, None, None] - 1)
    )
    dt = np.where(mask, dt, 0.0).astype(np.float32)
    hidden = np.maximum(dt[..., None] * W1[0] + b1, 0.0)
    kv = (hidden @ W2 + b2).reshape(Bn, Ln, KS, C, OUT)
    kv = np.where(mask[..., None, None], kv, 0.0)
    feat_g = features[:, idx_c]
    return np.einsum("blkc,blkco->blo", feat_g, kv).astype(np.float32)


def _build_in_maps(times, features, lengths, W1, W2):
    # Fold the (now linear) kernel-MLP into one 32x32 matrix.
    v16 = (np.maximum(W1[0], 0.0) @ W2).reshape(C, OUT).astype(np.float16)

    q = np.arange(128)
    r = np.arange(128)
    tt = np.arange(NT)

    in_maps = []
    for core in range(N_CORES):
        b, half = core // 2, core % 2
        start = half * HALF
        t_b = times[b]
        f_b = features[b]
        ln = int(lengths[b])

        Hs = start - 8 + STRIDE * tt                       # (NT,)
        jpos = Hs[:, None] + q[None, :]                    # (NT, 128) j per (t, q)
        ipos = Hs[:, None] + r[None, :]                    # (NT, 128) i per (t, r)

        # fTs[32*(t%4)+c, 128*(t//4)+q] = f_b[jpos[t,q], c] (0 out of range)
        jc = np.clip(jpos, 0, L - 1)
        fv = f_b[jc]                                       # (NT, 128, C)
        fv = np.where(((jpos >= 0) & (jpos < L))[..., None], fv, 0.0)
        imgA_h = np.zeros((96, IMGA_W), np.float16)
        for t in range(NT):
            s, g = t // 3, t % 3
            imgA_h[32 * s : 32 * s + 32, 128 * g : 128 * g + 128] = (
                fv[t].T.astype(np.float16)
            )
        for s in range(3):
            imgA_h[32 * s : 32 * s + 32, V_OFF : V_OFF + OUT] = v16

        # AT[q, t, r]: k = r - q in [1, 8]; value = t_b[i] - t_b[j], masked
        k = r[None, None, :] - q[:, None, None]            # (128, 1, 128)
        jq = jpos.T[:, :, None]                            # (128 q, NT, 1)
        ir = ipos[None, :, :]                              # (1, NT, 128)
        valid = (
            (k >= 1) & (k <= KS)
            & (jq >= 0) & (jq < ln)
            & (ir <= ln - 1) & (ir < L) & (ir >= 0)
        )                                                  # (128, NT, 128)
        ic = np.clip(ipos, 0, L - 1)                       # (NT, 128)
        dtv = t_b[ic][None, :, :] - t_b[jc].T[:, :, None]  # (128, NT, 128)
        at = np.where(valid, dtv, 0.0).astype(np.float16)  # (128 q, NT, 128 r)

        # banded split: out rows r<64 only need q<=62 (ATa, 64 partitions);
        # rows r>=64 need q in [56,126] (ATb, full partitions)
        inBa_h = np.ascontiguousarray(at[0:64, :, 0:64].reshape(64, ATA_W))
        inBb_h = np.ascontiguousarray(at[:, :, 64:128].reshape(128, ATB_W))

        in_maps.append({"inA": imgA_h, "inBa": inBa_h, "inBb": inBb_h})
    return in_maps


def kernel(times, features, lengths, W1, b1, W2, b2):
    times = np.asarray(times, dtype=np.float32)
    features = np.asarray(features, dtype=np.float32)
    lengths = np.asarray(lengths)
    W1 = np.asarray(W1, dtype=np.float32)
    b1 = np.asarray(b1, dtype=np.float32)
    W2 = np.asarray(W2, dtype=np.float32)
    b2 = np.asarray(b2, dtype=np.float32)

    if not _fast_path_ok(times, b1, b2):
        return _reference_fallback(times, features, lengths, W1, b1, W2, b2)

    from concourse.bass_utils import run_bass_kernel_spmd

    nc = _get_program()
    in_maps = _build_in_maps(times, features, lengths, W1, W2)
    res = run_bass_kernel_spmd(nc, in_maps, core_ids=list(range(N_CORES)))

    out = np.empty((B, L, OUT), np.float32)
    for core in range(N_CORES):
        b, half = core // 2, core % 2
        start = half * HALF
        r = res.results[core]["out"].astype(np.float32)   # (128, 288)
        for t in range(NT):
            n_t = min(STRIDE, HALF - STRIDE * t)
            if n_t <= 0:
                break
            out[b, start + STRIDE * t : start + STRIDE * t + n_t, :] = (
                r[8 : 8 + n_t, OUT * t : OUT * t + OUT]
            )
    return out
